# revision 29
# baseline (speedup 1.0000x reference)
"""Trainium2 Bass kernel for CrossAttention (B=4, L=2048, D=1024, 16 heads x 64).

Sharding: 8 cores = 4 batches x 2 head-halves (tensor parallel over heads).
Per core: Q = x1 @ Wq[:, half], K/V = x2 @ Wkv[:, half-cols], 8 heads of
attention, partial Y^T = Wo[half-rows]^T @ O^T.  Host sums the two partials
per batch, transposes, and adds the bias.

Dataflow (per core), everything feature-major ("transposed") so that the
softmax denominator reduction lands on the matmul contraction axis:
  x^T tiles   [D-part, L]     loaded with a strided (transposing) DMA
  Q^T = Wq^T x1^T  [IH, L]    lhsT = Wq tile (as stored), rhs = x1^T
  K^T = Wk^T x2^T  [IH, L]
  V    = x2 @ Wv   [L, IH]    lhsT = x1^T tile, rhs = Wv tile (row-major!)
  S^T_h tile = (K^T_h)^T-free matmul: lhsT=K^T_h [64,128], rhs=Q^T_h [64,512]
       two heads ride the PE array concurrently via base-partition 0/64
  E^T = exp(S^T / 8)          ACT engine, PSUM -> SBUF
  U_aug = [V_h | 1]^T-free accumulation over j: lhsT=V_aug [128,65], rhs=E^T
       row 64 of U_aug is the softmax denominator
  O^T_h = U_h * (1/denom) broadcast along partitions (gpsimd bcast + DVE mul)
  Y^T += Wo_tile^T @ O^T      accumulated over the inner half, DMA'd out

All matmuls run in float32r (full PE rate at N=512, ~tf32 precision).
"""

import numpy as np

# ---- problem constants (hardcoded; kernel.py must be self-contained) ----
B = 4
L = 2048
D = 1024
INNER = 1024
HEADS = 16
DH = 64
N_CORES = 8
IH = INNER // 2  # inner columns per core (8 heads)
SCALE = DH ** -0.5

_CACHE = {}


def _build_nc(L_=L, D_=D, IH_=IH, DH_=DH, LQB=512, XS=512, NOC=32, compile_=True, repeat=1):
    import concourse.bass as bass
    import concourse.tile as tile
    from concourse import bacc, mybir

    f32 = mybir.dt.float32
    f32r = mybir.dt.float32r
    P = 128
    NH = IH_ // DH_        # heads per core
    NPAIR = NH // 2        # head pairs
    NJT = L_ // P          # lk tiles
    NLQB = L_ // LQB       # lq blocks
    NDT = D_ // P          # D tiles
    NIT = IH_ // P         # inner-half tiles
    NXS = L_ // XS         # x slices
    VW = 2 * DH_ + 2       # v-pair width: [V_h0 | 1 | V_h1 | 1] = 130
    assert NH % 2 == 0 and L_ % LQB == 0 and LQB % P == 0

    nc = bacc.Bacc("TRN2", target_bir_lowering=False, debug=False)
    x1 = nc.declare_dram_parameter("x1t", [D_, L_], f32r, isOutput=False)
    x2 = nc.declare_dram_parameter("x2t", [D_, L_], f32r, isOutput=False)
    wq = nc.declare_dram_parameter("wq", [D_, IH_], f32r, isOutput=False)
    wk = nc.declare_dram_parameter("wk", [D_, IH_], f32r, isOutput=False)
    wv = nc.declare_dram_parameter("wv", [D_, IH_], f32r, isOutput=False)
    wo = nc.declare_dram_parameter("wo", [IH_, D_], f32r, isOutput=False)
    yt = nc.declare_dram_parameter("yt", [D_, L_], f32, isOutput=True)

    def r(ap):
        return ap

    import contextlib

    with tile.TileContext(nc) as tc:
        with (
            tc.For_i(0, repeat, 1) if repeat > 1 else contextlib.nullcontext(),
            tc.tile_pool(name="persist", bufs=1) as persist,
        ):
            # persistent SBUF tensors (one slot each; distinct names)
            kt = persist.tile([P, NIT * L_], f32r, name="kt")    # K^T blocks
            onesrow = persist.tile([P, NH * NOC], f32, name="onesrow")
            nc.vector.memset(onesrow, 1.0)
            qt = persist.tile([P, NIT * L_], f32r, name="qt")    # Q^T blocks
            vv = persist.tile([P, NJT * NH * (DH_ + NOC)], f32r, name="vv")
            VJ = NH * (DH_ + NOC)  # per-j-tile v width

            # ---- phases 1+2: projections ----
            with (
                tc.tile_pool(name="wts", bufs=1) as wts,
                tc.tile_pool(name="xts", bufs=2) as xts,
                tc.tile_pool(name="psum_acc", bufs=1, space="PSUM") as acc_pool,
            ):
                wkt = wts.tile([P, NDT * IH_], f32r, name="wkt")
                wvt = wts.tile([P, NDT * IH_], f32r, name="wvt")
                wqt = wts.tile([P, NDT * IH_], f32r, name="wqt")

                def load_weights():
                    for d in range(NDT):
                        if d > 0:
                            nc.sync.dma_start(
                                out=wkt[:, d * IH_:(d + 1) * IH_],
                                in_=wk[d * P:(d + 1) * P, :])
                        nc.sync.dma_start(
                            out=wvt[:, d * IH_:(d + 1) * IH_],
                            in_=wv[d * P:(d + 1) * P, :])
                        nc.sync.dma_start(
                            out=wqt[:, d * IH_:(d + 1) * IH_],
                            in_=wq[d * P:(d + 1) * P, :])

                # ones columns of the V_aug layout (DVE cast-copy from the
                # f32 ones tile; memset cannot write f32r directly)
                for j in range(NJT):
                    ones_ap = (vv[:, j * VJ:(j + 1) * VJ]
                               .rearrange("p (h c) -> p h c", c=DH_ + NOC)[:, :, DH_:])
                    nc.vector.tensor_copy(ones_ap, onesrow.rearrange(
                        "p (h c) -> p h c", c=NOC))

                def load_xt_slice(xt_dram, s):
                    tiles = []
                    for d in range(NDT):
                        xt_t = xts.tile([P, XS], f32r, name="xt",
                                        tag=f"xt{d}",
                                        bufs=2 if d < NDT // 2 else 1)
                        nc.sync.dma_start(
                            out=xt_t,
                            in_=xt_dram[d * P:(d + 1) * P, s * XS:(s + 1) * XS])
                        tiles.append(xt_t)
                    return tiles

                # phase 1: K^T and V from x2 (first weight tile, then x
                # slices, then the remaining weights, so PE starts as early
                # as possible). d is the outer loop so each xt[d] tile is
                # consumed the moment its DMA lands.
                nc.sync.dma_start(out=wkt[:, 0:IH_], in_=wk[0:P, :])
                for s in range(NXS):
                    xt = load_xt_slice(x2, s)
                    if s == 0:
                        load_weights()
                    pks = [acc_pool.tile([P, XS], f32, name="pk",
                                         tag=f"pk{m}", bufs=1)
                           for m in range(NIT)]
                    pvs = [acc_pool.tile([P, IH_], f32, name="pv",
                                         tag=f"pv{t}", bufs=1)
                           for t in range(XS // P)]
                    for d in range(NDT):
                        for m in range(NIT):
                            nc.tensor.matmul(
                                pks[m],
                                lhsT=wkt[:, d * IH_ + m * P: d * IH_ + (m + 1) * P],
                                rhs=xt[d],
                                start=(d == 0), stop=(d == NDT - 1))
                        for t in range(XS // P):
                            nc.tensor.matmul(
                                pvs[t],
                                lhsT=xt[d][:, t * P:(t + 1) * P],
                                rhs=wvt[:, d * IH_:(d + 1) * IH_],
                                start=(d == 0), stop=(d == NDT - 1))
                    for m in range(NIT):
                        nc.vector.tensor_copy(
                            kt[:, m * L_ + s * XS: m * L_ + (s + 1) * XS],
                            pks[m])
                    for t in range(XS // P):
                        j = s * (XS // P) + t
                        dst = (vv[:, j * VJ:(j + 1) * VJ]
                               .rearrange("p (h c) -> p h c", c=DH_ + NOC)[:, :, :DH_])
                        srcv = pvs[t].rearrange("p (h c) -> p h c", c=DH_)
                        nc.vector.tensor_copy(dst, srcv)

                # phase 2: Q^T from x1
                for s in range(NXS):
                    xt = load_xt_slice(x1, s)
                    pqs = [acc_pool.tile([P, XS], f32, name="pq",
                                         tag=f"pk{m}", bufs=1)
                           for m in range(NIT)]
                    for d in range(NDT):
                        for m in range(NIT):
                            nc.tensor.matmul(
                                pqs[m],
                                lhsT=wqt[:, d * IH_ + m * P: d * IH_ + (m + 1) * P],
                                rhs=xt[d],
                                start=(d == 0), stop=(d == NDT - 1))
                    for m in range(NIT):
                        nc.vector.tensor_copy(
                            qt[:, m * L_ + s * XS: m * L_ + (s + 1) * XS],
                            pqs[m])

            # ---- phase 3: attention + output projection ----
            with (
                tc.tile_pool(name="wo_pool", bufs=1) as wo_pool,
                tc.tile_pool(name="spsum", bufs=2, space="PSUM") as s_pool,
                tc.tile_pool(name="upsum", bufs=4, space="PSUM") as u_pool,
                tc.tile_pool(name="ets", bufs=3) as ets,
                tc.tile_pool(name="smalls", bufs=4) as smalls,
                tc.tile_pool(name="ot_pool", bufs=2) as ot_pool,
                tc.tile_pool(name="youts", bufs=3) as youts,
            ):
                wot = wo_pool.tile([P, NIT * D_], f32r, name="wot")
                for it in range(NIT):
                    nc.sync.dma_start(
                        out=wot[:, it * D_:(it + 1) * D_],
                        in_=wo[it * P:(it + 1) * P, :])

                def emit_yproj(ot_prev, lqb_prev, dsub):
                    py = u_pool.tile([P, LQB], f32, name="py", tag="u")
                    for it in range(NIT):
                        nc.tensor.matmul(
                            py,
                            lhsT=wot[:, it * D_ + dsub * P: it * D_ + (dsub + 1) * P],
                            rhs=ot_prev[:, it * LQB:(it + 1) * LQB],
                            start=(it == 0), stop=(it == NIT - 1))
                    yo = youts.tile([P, LQB], f32, name="yo")
                    nc.vector.tensor_copy(yo, py)
                    nc.sync.dma_start(
                        out=yt[dsub * P:(dsub + 1) * P,
                               lqb_prev * LQB:(lqb_prev + 1) * LQB],
                        in_=yo)

                ydefer = []  # (ot, lqb, dsub) of the previous block

                for lqb in range(NLQB):
                    ot = ot_pool.tile([P, NIT * LQB], f32r, name="ot")
                    for hp in range(NPAIR):
                        u0 = u_pool.tile([P, LQB], f32, name="u0", tag="u")
                        u1 = u_pool.tile([P, LQB], f32, name="u1", tag="u")
                        for j in range(NJT):
                            st = s_pool.tile([P, 2 * LQB], f32, name="st",
                                             tag="st")
                            # head pair rides PE rows 0-63 / 64-127
                            nc.tensor.matmul(
                                st[:, 0:LQB],
                                lhsT=kt[0:DH_, hp * L_ + j * P: hp * L_ + (j + 1) * P],
                                rhs=qt[0:DH_, hp * L_ + lqb * LQB: hp * L_ + lqb * LQB + LQB],
                                start=True, stop=True)
                            nc.tensor.matmul(
                                st[:, LQB:2 * LQB],
                                lhsT=kt[DH_:2 * DH_, hp * L_ + j * P: hp * L_ + (j + 1) * P],
                                rhs=qt[DH_:2 * DH_, hp * L_ + lqb * LQB: hp * L_ + lqb * LQB + LQB],
                                start=True, stop=True)
                            et = ets.tile([P, 2 * LQB], f32r, name="et")
                            nc.scalar.activation(
                                et, st, mybir.ActivationFunctionType.Exp,
                                scale=float(SCALE))
                            for hh, u in ((0, u0), (1, u1)):
                                h = 2 * hp + hh
                                nc.tensor.matmul(
                                    u[0:DH_ + NOC, :],
                                    lhsT=vv[:, j * VJ + h * (DH_ + NOC): j * VJ + (h + 1) * (DH_ + NOC)],
                                    rhs=et[:, hh * LQB:(hh + 1) * LQB],
                                    start=(j == 0), stop=(j == NJT - 1))
                        for hh, u in ((0, u0), (1, u1)):
                            # all-DVE normalization: recip of the (replicated)
                            # denominator rows, quadrant-broadcast via
                            # stream_shuffle, fused multiply evicts O^T
                            rcp = smalls.tile([NOC, LQB], f32, name="rcp",
                                              tag=f"rcp{hh}")
                            nc.vector.reciprocal(rcp, u[DH_:DH_ + NOC, :])
                            rb = smalls.tile([DH_, LQB], f32, name="rb",
                                             tag=f"rb{hh}")
                            zmask = [0] * 32
                            nc.vector.stream_shuffle(rb[0:32, :], rcp, zmask)
                            nc.vector.stream_shuffle(rb[32:64, :], rcp, zmask)
                            dst = ot[hh * DH_:(hh + 1) * DH_,
                                     hp * LQB:(hp + 1) * LQB]
                            nc.vector.tensor_mul(dst, u[0:DH_, :], rb)
                        # two output-projection column groups of the previous
                        # lq block ride along to fill PE gaps
                        for _ in range(2):
                            if ydefer:
                                emit_yproj(*ydefer.pop(0))
                    ydefer.extend((ot, lqb, dsub) for dsub in range(D_ // P))
                while ydefer:
                    emit_yproj(*ydefer.pop(0))
    if compile_:
        nc.compile()
    return nc


def _get_nc():
    if "nc" not in _CACHE:
        _CACHE["nc"] = _build_nc()
    return _CACHE["nc"]


def kernel(x1, x2, Wq, Wkv, Wo, bo):
    import sys
    if "/opt/trn_rl_repo" not in sys.path:
        sys.path.insert(0, "/opt/trn_rl_repo")
    from concourse.bass_utils import run_bass_kernel_spmd

    x1 = np.asarray(x1, dtype=np.float32)
    x2 = np.asarray(x2, dtype=np.float32)
    Wq = np.asarray(Wq, dtype=np.float32)
    Wkv = np.asarray(Wkv, dtype=np.float32)
    Wo = np.asarray(Wo, dtype=np.float32)
    bo = np.asarray(bo, dtype=np.float32)

    nc = _get_nc()
    res = run_bass_kernel_spmd(nc, _make_in_maps(x1, x2, Wq, Wkv, Wo),
                               list(range(N_CORES)))
    return _gather(res.results, bo)


def _make_in_maps(x1, x2, Wq, Wkv, Wo):
    x1t = [np.ascontiguousarray(x1[b].T) for b in range(B)]
    x2t = [np.ascontiguousarray(x2[b].T) for b in range(B)]
    in_maps = []
    for c in range(N_CORES):
        b, t = c // 2, c % 2
        in_maps.append({
            "x1t": x1t[b],
            "x2t": x2t[b],
            "wq": np.ascontiguousarray(Wq[:, t * IH:(t + 1) * IH]),
            "wk": np.ascontiguousarray(Wkv[:, t * IH:(t + 1) * IH]),
            "wv": np.ascontiguousarray(Wkv[:, INNER + t * IH: INNER + (t + 1) * IH]),
            "wo": np.ascontiguousarray(Wo[t * IH:(t + 1) * IH, :]),
        })
    return in_maps


def _gather(outs, bo):
    y = np.empty((B, L, D), dtype=np.float32)
    for b in range(B):
        y[b] = (outs[2 * b]["yt"] + outs[2 * b + 1]["yt"]).T + bo
    return y


# revision 33
# speedup vs baseline: 1.0019x; 1.0019x over previous
"""Trainium2 Bass kernel for CrossAttention (B=4, L=2048, D=1024, 16 heads x 64).

Sharding: 8 cores = 4 batches x 2 head-halves (tensor parallel over heads).
Per core: Q = x1 @ Wq[:, half], K/V = x2 @ Wkv[:, half-cols], 8 heads of
attention, partial Y^T = Wo[half-rows]^T @ O^T.  Host sums the two partials
per batch, transposes, and adds the bias.

Dataflow (per core), everything feature-major ("transposed") so that the
softmax denominator reduction lands on the matmul contraction axis:
  x^T tiles   [D-part, L]     loaded with a strided (transposing) DMA
  Q^T = Wq^T x1^T  [IH, L]    lhsT = Wq tile (as stored), rhs = x1^T
  K^T = Wk^T x2^T  [IH, L]
  V    = x2 @ Wv   [L, IH]    lhsT = x1^T tile, rhs = Wv tile (row-major!)
  S^T_h tile = (K^T_h)^T-free matmul: lhsT=K^T_h [64,128], rhs=Q^T_h [64,512]
       two heads ride the PE array concurrently via base-partition 0/64
  E^T = exp(S^T / 8)          ACT engine, PSUM -> SBUF
  U_aug = [V_h | 1]^T-free accumulation over j: lhsT=V_aug [128,65], rhs=E^T
       row 64 of U_aug is the softmax denominator
  O^T_h = U_h * (1/denom) broadcast along partitions (gpsimd bcast + DVE mul)
  Y^T += Wo_tile^T @ O^T      accumulated over the inner half, DMA'd out

All matmuls run in float32r (full PE rate at N=512, ~tf32 precision).
"""

import numpy as np

# ---- problem constants (hardcoded; kernel.py must be self-contained) ----
B = 4
L = 2048
D = 1024
INNER = 1024
HEADS = 16
DH = 64
N_CORES = 8
IH = INNER // 2  # inner columns per core (8 heads)
SCALE = DH ** -0.5

_CACHE = {}


def _build_nc(L_=L, D_=D, IH_=IH, DH_=DH, LQB=512, XS=512, NOC=32, compile_=True, repeat=1):
    import concourse.bass as bass
    import concourse.tile as tile
    from concourse import bacc, mybir

    f32 = mybir.dt.float32
    f32r = mybir.dt.float32r
    P = 128
    NH = IH_ // DH_        # heads per core
    NPAIR = NH // 2        # head pairs
    NJT = L_ // P          # lk tiles
    NLQB = L_ // LQB       # lq blocks
    NDT = D_ // P          # D tiles
    NIT = IH_ // P         # inner-half tiles
    NXS = L_ // XS         # x slices
    VW = 2 * DH_ + 2       # v-pair width: [V_h0 | 1 | V_h1 | 1] = 130
    assert NH % 2 == 0 and L_ % LQB == 0 and LQB % P == 0

    nc = bacc.Bacc("TRN2", target_bir_lowering=False, debug=False)
    x1 = nc.declare_dram_parameter("x1t", [D_, L_], f32r, isOutput=False)
    x2 = nc.declare_dram_parameter("x2t", [D_, L_], f32r, isOutput=False)
    wq = nc.declare_dram_parameter("wq", [D_, IH_], f32r, isOutput=False)
    wk = nc.declare_dram_parameter("wk", [D_, IH_], f32r, isOutput=False)
    wv = nc.declare_dram_parameter("wv", [D_, IH_], f32r, isOutput=False)
    wo = nc.declare_dram_parameter("wo", [IH_, D_], f32r, isOutput=False)
    yt = nc.declare_dram_parameter("yt", [D_, L_], f32, isOutput=True)

    def r(ap):
        return ap

    import contextlib

    with tile.TileContext(nc) as tc:
        with (
            tc.For_i(0, repeat, 1) if repeat > 1 else contextlib.nullcontext(),
            tc.tile_pool(name="persist", bufs=1) as persist,
        ):
            # persistent SBUF tensors (one slot each; distinct names)
            kt = persist.tile([P, NIT * L_], f32r, name="kt")    # K^T blocks
            onesrow = persist.tile([P, NH * NOC], f32, name="onesrow")
            nc.vector.memset(onesrow, 1.0)
            qt = persist.tile([P, NIT * L_], f32r, name="qt")    # Q^T blocks
            vv = persist.tile([P, NJT * NH * (DH_ + NOC)], f32r, name="vv")
            VJ = NH * (DH_ + NOC)  # per-j-tile v width

            # ---- phases 1+2: projections ----
            with (
                tc.tile_pool(name="wts", bufs=1) as wts,
                tc.tile_pool(name="xts", bufs=2) as xts,
                tc.tile_pool(name="psum_acc", bufs=1, space="PSUM") as acc_pool,
            ):
                wkt = wts.tile([P, NDT * IH_], f32r, name="wkt")
                wvt = wts.tile([P, NDT * IH_], f32r, name="wvt")
                wqt = wts.tile([P, NDT * IH_], f32r, name="wqt")

                def load_weights():
                    for d in range(NDT):
                        if d > 0:
                            nc.sync.dma_start(
                                out=wkt[:, d * IH_:(d + 1) * IH_],
                                in_=wk[d * P:(d + 1) * P, :])
                        nc.sync.dma_start(
                            out=wvt[:, d * IH_:(d + 1) * IH_],
                            in_=wv[d * P:(d + 1) * P, :])
                        nc.sync.dma_start(
                            out=wqt[:, d * IH_:(d + 1) * IH_],
                            in_=wq[d * P:(d + 1) * P, :])

                # ones columns of the V_aug layout (DVE cast-copy from the
                # f32 ones tile; memset cannot write f32r directly)
                for j in range(NJT):
                    ones_ap = (vv[:, j * VJ:(j + 1) * VJ]
                               .rearrange("p (h c) -> p h c", c=DH_ + NOC)[:, :, DH_:])
                    nc.vector.tensor_copy(ones_ap, onesrow.rearrange(
                        "p (h c) -> p h c", c=NOC))

                def load_xt_slice(xt_dram, s):
                    tiles = []
                    for d in range(NDT):
                        xt_t = xts.tile([P, XS], f32r, name="xt",
                                        tag=f"xt{d}",
                                        bufs=2 if d < NDT // 2 else 1)
                        nc.sync.dma_start(
                            out=xt_t,
                            in_=xt_dram[d * P:(d + 1) * P, s * XS:(s + 1) * XS])
                        tiles.append(xt_t)
                    return tiles

                # phase 1: K^T and V from x2 (first weight tile, then x
                # slices, then the remaining weights, so PE starts as early
                # as possible). d is the outer loop so each xt[d] tile is
                # consumed the moment its DMA lands.
                nc.sync.dma_start(out=wkt[:, 0:IH_], in_=wk[0:P, :])
                for s in range(NXS):
                    xt = load_xt_slice(x2, s)
                    if s == 0:
                        load_weights()
                    pks = [acc_pool.tile([P, XS], f32, name="pk",
                                         tag=f"pk{m}", bufs=1)
                           for m in range(NIT)]
                    pvs = [acc_pool.tile([P, IH_], f32, name="pv",
                                         tag=f"pv{t}", bufs=1)
                           for t in range(XS // P)]
                    for d in range(NDT):
                        for m in range(NIT):
                            nc.tensor.matmul(
                                pks[m],
                                lhsT=wkt[:, d * IH_ + m * P: d * IH_ + (m + 1) * P],
                                rhs=xt[d],
                                start=(d == 0), stop=(d == NDT - 1))
                        for t in range(XS // P):
                            nc.tensor.matmul(
                                pvs[t],
                                lhsT=xt[d][:, t * P:(t + 1) * P],
                                rhs=wvt[:, d * IH_:(d + 1) * IH_],
                                start=(d == 0), stop=(d == NDT - 1))
                    for m in range(NIT):
                        nc.vector.tensor_copy(
                            kt[:, m * L_ + s * XS: m * L_ + (s + 1) * XS],
                            pks[m])
                    for t in range(XS // P):
                        j = s * (XS // P) + t
                        dst = (vv[:, j * VJ:(j + 1) * VJ]
                               .rearrange("p (h c) -> p h c", c=DH_ + NOC)[:, :, :DH_])
                        srcv = pvs[t].rearrange("p (h c) -> p h c", c=DH_)
                        nc.vector.tensor_copy(dst, srcv)

                # phase 2: Q^T from x1
                for s in range(NXS):
                    xt = load_xt_slice(x1, s)
                    pqs = [acc_pool.tile([P, XS], f32, name="pq",
                                         tag=f"pk{m}", bufs=1)
                           for m in range(NIT)]
                    for d in range(NDT):
                        for m in range(NIT):
                            nc.tensor.matmul(
                                pqs[m],
                                lhsT=wqt[:, d * IH_ + m * P: d * IH_ + (m + 1) * P],
                                rhs=xt[d],
                                start=(d == 0), stop=(d == NDT - 1))
                    for m in range(NIT):
                        nc.vector.tensor_copy(
                            qt[:, m * L_ + s * XS: m * L_ + (s + 1) * XS],
                            pqs[m])

            # ---- phase 3: attention + output projection ----
            with (
                tc.tile_pool(name="wo_pool", bufs=1) as wo_pool,
                tc.tile_pool(name="spsum", bufs=2, space="PSUM") as s_pool,
                tc.tile_pool(name="upsum", bufs=4, space="PSUM") as u_pool,
                tc.tile_pool(name="ets", bufs=3) as ets,
                tc.tile_pool(name="smalls", bufs=4) as smalls,
                tc.tile_pool(name="ot_pool", bufs=2) as ot_pool,
                tc.tile_pool(name="youts", bufs=3) as youts,
            ):
                wot = wo_pool.tile([P, NIT * D_], f32r, name="wot")
                for it in range(NIT):
                    nc.sync.dma_start(
                        out=wot[:, it * D_:(it + 1) * D_],
                        in_=wo[it * P:(it + 1) * P, :])

                def emit_yproj(ot_prev, lqb_prev, dsub):
                    py = u_pool.tile([P, LQB], f32, name="py", tag="u")
                    for it in range(NIT):
                        nc.tensor.matmul(
                            py,
                            lhsT=wot[:, it * D_ + dsub * P: it * D_ + (dsub + 1) * P],
                            rhs=ot_prev[:, it * LQB:(it + 1) * LQB],
                            start=(it == 0), stop=(it == NIT - 1))
                    yo = youts.tile([P, LQB], f32, name="yo")
                    nc.vector.tensor_copy(yo, py)
                    nc.sync.dma_start(
                        out=yt[dsub * P:(dsub + 1) * P,
                               lqb_prev * LQB:(lqb_prev + 1) * LQB],
                        in_=yo)

                ydefer = []  # (ot, lqb, dsub) of the previous block

                for lqb in range(NLQB):
                    ot = ot_pool.tile([P, NIT * LQB], f32r, name="ot")
                    for hp in range(NPAIR):
                        u0 = u_pool.tile([P, LQB], f32, name="u0", tag="u")
                        u1 = u_pool.tile([P, LQB], f32, name="u1", tag="u")
                        for j in range(NJT):
                            st = s_pool.tile([P, 2 * LQB], f32, name="st",
                                             tag="st")
                            # head pair rides PE rows 0-63 / 64-127
                            nc.tensor.matmul(
                                st[:, 0:LQB],
                                lhsT=kt[0:DH_, hp * L_ + j * P: hp * L_ + (j + 1) * P],
                                rhs=qt[0:DH_, hp * L_ + lqb * LQB: hp * L_ + lqb * LQB + LQB],
                                start=True, stop=True)
                            nc.tensor.matmul(
                                st[:, LQB:2 * LQB],
                                lhsT=kt[DH_:2 * DH_, hp * L_ + j * P: hp * L_ + (j + 1) * P],
                                rhs=qt[DH_:2 * DH_, hp * L_ + lqb * LQB: hp * L_ + lqb * LQB + LQB],
                                start=True, stop=True)
                            et = ets.tile([P, 2 * LQB], f32r, name="et")
                            nc.scalar.activation(
                                et, st, mybir.ActivationFunctionType.Exp,
                                scale=float(SCALE))
                            for hh, u in ((0, u0), (1, u1)):
                                h = 2 * hp + hh
                                nc.tensor.matmul(
                                    u[0:DH_ + NOC, :],
                                    lhsT=vv[:, j * VJ + h * (DH_ + NOC): j * VJ + (h + 1) * (DH_ + NOC)],
                                    rhs=et[:, hh * LQB:(hh + 1) * LQB],
                                    start=(j == 0), stop=(j == NJT - 1))
                        for hh, u in ((0, u0), (1, u1)):
                            # all-DVE normalization: recip of the (replicated)
                            # denominator rows, quadrant-broadcast via
                            # stream_shuffle, fused multiply evicts O^T
                            rcp = smalls.tile([NOC, LQB], f32, name="rcp",
                                              tag=f"rcp{hh}")
                            nc.vector.reciprocal(rcp, u[DH_:DH_ + NOC, :])
                            rb = smalls.tile([DH_, LQB], f32, name="rb",
                                             tag=f"rb{hh}")
                            zmask = [0] * 32
                            nc.vector.stream_shuffle(rb[0:32, :], rcp, zmask)
                            nc.vector.stream_shuffle(rb[32:64, :], rcp, zmask)
                            dst = ot[hh * DH_:(hh + 1) * DH_,
                                     hp * LQB:(hp + 1) * LQB]
                            nc.vector.tensor_mul(dst, u[0:DH_, :], rb)
                        # two output-projection column groups of the previous
                        # lq block ride along to fill PE gaps
                        for _ in range(3):
                            if ydefer:
                                emit_yproj(*ydefer.pop(0))
                    ydefer.extend((ot, lqb, dsub) for dsub in range(D_ // P))
                while ydefer:
                    emit_yproj(*ydefer.pop(0))
    if compile_:
        nc.compile()
    return nc


def _get_nc():
    if "nc" not in _CACHE:
        _CACHE["nc"] = _build_nc()
    return _CACHE["nc"]


def kernel(x1, x2, Wq, Wkv, Wo, bo):
    import sys
    if "/opt/trn_rl_repo" not in sys.path:
        sys.path.insert(0, "/opt/trn_rl_repo")
    from concourse.bass_utils import run_bass_kernel_spmd

    x1 = np.asarray(x1, dtype=np.float32)
    x2 = np.asarray(x2, dtype=np.float32)
    Wq = np.asarray(Wq, dtype=np.float32)
    Wkv = np.asarray(Wkv, dtype=np.float32)
    Wo = np.asarray(Wo, dtype=np.float32)
    bo = np.asarray(bo, dtype=np.float32)

    nc = _get_nc()
    res = run_bass_kernel_spmd(nc, _make_in_maps(x1, x2, Wq, Wkv, Wo),
                               list(range(N_CORES)))
    return _gather(res.results, bo)


def _make_in_maps(x1, x2, Wq, Wkv, Wo):
    x1t = [np.ascontiguousarray(x1[b].T) for b in range(B)]
    x2t = [np.ascontiguousarray(x2[b].T) for b in range(B)]
    in_maps = []
    for c in range(N_CORES):
        b, t = c // 2, c % 2
        in_maps.append({
            "x1t": x1t[b],
            "x2t": x2t[b],
            "wq": np.ascontiguousarray(Wq[:, t * IH:(t + 1) * IH]),
            "wk": np.ascontiguousarray(Wkv[:, t * IH:(t + 1) * IH]),
            "wv": np.ascontiguousarray(Wkv[:, INNER + t * IH: INNER + (t + 1) * IH]),
            "wo": np.ascontiguousarray(Wo[t * IH:(t + 1) * IH, :]),
        })
    return in_maps


def _gather(outs, bo):
    y = np.empty((B, L, D), dtype=np.float32)
    for b in range(B):
        y[b] = (outs[2 * b]["yt"] + outs[2 * b + 1]["yt"]).T + bo
    return y
